# revision 4
# baseline (speedup 1.0000x reference)
"""AttnGatedCRFV2 loss on 8 Trainium2 NeuronCores.

Math (identical to the reference, no [B,HW,HW] intermediates):
    m    = (1 - eye(HW)) * kernel_xy_mask          (diag folded on host)
    G_b  = kernel_b * m                            (folded on host, fp8 e4m3)
    loss = (1/(HW*B)) * [ sum_b <G_b, 1 - 2*Y_b Y_b^T> + <m, sum_b Y_b Y_b^T> ]
         = (1/(HW*B)) * [ sum_b sum_j ( R1_b[4,j] + sum_c -2*y_b[c,j]*R1_b[c,j] )
                          + sum_j sum_ch wc[ch,j]*R3[ch,j] ]
    where R1_b = [Y_b | 1]^T G_b  (rows c<4 and the ones row 4),
          R3   = Ystack^T m        (Ystack packs all 32 (b,c) channels).

Sharding: j-columns split across 8 cores (288 cols each). Each core reads
G[:, :, j0:j0+288] (5.31 MB in fp8) once; host sums 8 scalar partials.

Device pipeline per core (all heavy traffic fp8):
  - G arrives host-packed as [B, 128, 9, 2, 288]: DoubleRow pair-group s
    holds i-rows 256s+128t+p.  3 chunked DMAs per batch over 3 queues.
  - PE: 9 fp8 DoubleRow matmuls per batch (K=256 each) accumulate
    R1_b = Yext_b^T G_b into a [8, 288] PSUM bank; the resident mask tile
    feeds an identical 9-matmul chain for R3 ([32, 288]).
  - DVE: one fused affine_mul_reduce per bank: accum[p] = sum_j R1[p,j]*W[p,j]
    straight from PSUM (no ACT copy), landing in per-batch columns of a
    small SBUF accumulator.
  - tail: row-reduce the accumulator, two tiny f32 matmuls against a
    SCALE-filled vector produce the [1,1] partial, DMA out.

Host prep is layout/dtype only: fold mask & diag into G, cast fp8/bf16,
pack the DoubleRow layout, fold -2*y into stage-2 weights (as the previous
fp32 baseline already did).  fp8 e4m3 quantization of G/Y changes the loss
by ~2e-6 relative (tolerance 2e-2): errors average out over 42M terms.
"""
import numpy as np
import ml_dtypes
from contextlib import ExitStack

B, C, H, W = 8, 4, 48, 48
HW = H * W                    # 2304
NCORES = 8
JW = HW // NCORES             # 288 j-columns per core
NS = 9                        # DoubleRow pair groups (2304 = 9*2*128)
SCALE = 1.0 / float(HW * B)

_BUILT = None
LAST_RESULT = None


def _build(loop_n=None, gbufs=8, nchunks=3):
    from concourse import bacc, tile, mybir

    f32, bf16 = mybir.dt.float32, mybir.dt.bfloat16
    f8 = mybir.dt.float8e4
    DR = mybir.MatmulPerfMode.DoubleRow

    nc = bacc.Bacc("TRN2", target_bir_lowering=False, debug=False,
                   num_devices=NCORES)

    g_ap = nc.dram_tensor("gk", [B, 128, NS, 2, JW], f8, kind="ExternalInput").ap()
    m_ap = nc.dram_tensor("mk", [128, NS, 2, JW], f8, kind="ExternalInput").ap()
    yp_ap = nc.dram_tensor("yp", [B, 128, NS, 2, 16], f8, kind="ExternalInput").ap()
    ys_ap = nc.dram_tensor("ys", [128, NS, 2, 32], f8, kind="ExternalInput").ap()
    wv_ap = nc.dram_tensor("wv", [B, 16, JW], bf16, kind="ExternalInput").ap()
    wc_ap = nc.dram_tensor("wc", [32, JW], bf16, kind="ExternalInput").ap()
    sc_ap = nc.dram_tensor("sc", [32, 1], f32, kind="ExternalInput").ap()
    out_ap = nc.dram_tensor("partial", [1, 1], f32, kind="ExternalOutput").ap()

    qs = [nc.gpsimd, nc.sync, nc.scalar]

    with tile.TileContext(nc) as tc, ExitStack() as ctx:
        consts = ctx.enter_context(tc.tile_pool(name="consts", bufs=1))
        gp = ctx.enter_context(tc.tile_pool(name="gp", bufs=gbufs))
        scr = ctx.enter_context(tc.tile_pool(name="scr", bufs=2))
        smp = ctx.enter_context(tc.tile_pool(name="smp", bufs=2))
        psb = ctx.enter_context(tc.tile_pool(name="psb", bufs=4, space="PSUM"))
        psc = ctx.enter_context(tc.tile_pool(name="psc", bufs=2, space="PSUM"))
        psf = ctx.enter_context(tc.tile_pool(name="psf", bufs=2, space="PSUM"))

        # ---- resident constants ----
        mt = consts.tile([128, NS, 2, JW], f8, tag="mt")
        nc.gpsimd.dma_start(mt[:], m_ap[:])
        yp = {}
        for b in range(B):
            t = consts.tile([128, NS, 2, 16], f8, tag=f"yp{b}")
            nc.sync.dma_start(t[:], yp_ap[b])
            yp[b] = t
        ys = consts.tile([128, NS, 2, 32], f8, tag="ys")
        nc.sync.dma_start(ys[:], ys_ap[:])
        wv = {}
        for b in range(B):
            t = consts.tile([16, JW], bf16, tag=f"wv{b}")
            nc.sync.dma_start(t[:], wv_ap[b])
            wv[b] = t
        wct = consts.tile([32, JW], bf16, tag="wct")
        nc.sync.dma_start(wct[:], wc_ap[:])
        sct = consts.tile([32, 1], f32, tag="sct")
        nc.sync.dma_start(sct[:], sc_ap[:])

        csz = NS // nchunks

        def body():
            # G tile DMAs first so all queues start streaming immediately
            gt = {}
            for b in range(B):
                t = gp.tile([128, NS, 2, JW], f8, tag="g", name=f"g{b}")
                for ch in range(nchunks):
                    s0 = ch * csz
                    qs[(b * nchunks + ch) % len(qs)].dma_start(
                        t[:, s0 : s0 + csz, :, :], g_ap[b, :, s0 : s0 + csz, :, :])
                gt[b] = t

            # C chain on the resident mask: R3 = Ystack^T m
            pc = psc.tile([32, JW], f32, tag="pc", name="pc")
            for s in range(NS):
                nc.tensor.matmul(out=pc[:, :], lhsT=ys[:, s, :, :],
                                 rhs=mt[:, s, :, :], perf_mode=DR,
                                 start=(s == 0), stop=(s == NS - 1))
            outc = scr.tile([32, JW], bf16, tag="outc", name="outc")
            accC = smp.tile([32, 1], f32, tag="accC", name="accC")
            nc.vector.affine_mul_reduce(out=outc[:], accum_out=accC[:],
                                        in0=pc[:, :], in1=wct[:], scale=1.0,
                                        bias=0.0)

            # main: R1_b = Yext_b^T G_b, then fused product+reduce per batch
            accA = smp.tile([16, B], f32, tag="accA", name="accA")
            for b in range(B):
                pb = psb.tile([16, JW], f32, tag="pb", name=f"pb{b}")
                for s in range(NS):
                    nc.tensor.matmul(out=pb[:, :], lhsT=yp[b][:, s, :, :],
                                     rhs=gt[b][:, s, :, :], perf_mode=DR,
                                     start=(s == 0), stop=(s == NS - 1))
                outb = scr.tile([16, JW], bf16, tag="outb", name=f"outb{b}")
                nc.vector.affine_mul_reduce(out=outb[:], accum_out=accA[:, b : b + 1],
                                            in0=pb[:, :], in1=wv[b][:], scale=1.0,
                                            bias=0.0)

            # tail: reduce accA cols, contract both accs against SCALE vector
            accAr = smp.tile([16, 1], f32, tag="accAr", name="accAr")
            nc.vector.tensor_reduce(out=accAr[:], in_=accA[:],
                                    axis=mybir.AxisListType.X,
                                    op=mybir.AluOpType.add)
            fin = psf.tile([1, 1], f32, tag="fin", name="fin")
            nc.tensor.matmul(out=fin[:], lhsT=accAr[:], rhs=sct[0:16, :],
                             start=True, stop=False)
            nc.tensor.matmul(out=fin[:], lhsT=accC[:], rhs=sct[:, :],
                             start=False, stop=True)
            sc_out = smp.tile([1, 1], f32, tag="sc_out", name="sc_out")
            nc.scalar.copy(sc_out[:], fin[:])
            nc.sync.dma_start(out_ap[:, :], sc_out[:])

        if loop_n is None:
            body()
        else:
            with tc.For_i(0, loop_n, 1):
                body()

    nc.compile()
    return nc


def _prep_inputs(y_hat_softmax, kern, mask):
    e4 = ml_dtypes.float8_e4m3
    bf = ml_dtypes.bfloat16
    y = np.ascontiguousarray(np.asarray(y_hat_softmax, np.float32)).reshape(B, C, HW)
    kern = np.asarray(kern, np.float32)
    m = np.asarray(mask, np.float32) * (1.0 - np.eye(HW, dtype=np.float32))

    G8 = (kern * m[None]).astype(e4)                 # [B, HW, HW]
    m8 = m.astype(e4)                                # exact 0/1 in fp8
    yk = y.transpose(0, 2, 1)                        # [B, HW, C]
    yx = np.zeros((B, HW, 16), np.float32)
    yx[:, :, :C] = yk
    yx[:, :, 4] = 1.0
    yp_full = np.ascontiguousarray(
        yx.reshape(B, NS, 2, 128, 16).transpose(0, 3, 1, 2, 4)).astype(e4)
    yst = np.ascontiguousarray(yk.transpose(1, 0, 2)).reshape(HW, 32)
    ys_full = np.ascontiguousarray(
        yst.reshape(NS, 2, 128, 32).transpose(2, 0, 1, 3)).astype(e4)
    wcf = y.reshape(32, HW)
    sc = np.full((32, 1), SCALE, np.float32)

    in_maps = []
    for cid in range(NCORES):
        j0 = JW * cid
        sl = slice(j0, j0 + JW)
        gk = np.ascontiguousarray(
            G8[:, :, sl].reshape(B, NS, 2, 128, JW).transpose(0, 3, 1, 2, 4))
        mk = np.ascontiguousarray(
            m8[:, sl].reshape(NS, 2, 128, JW).transpose(2, 0, 1, 3))
        wv = np.zeros((B, 16, JW), np.float32)
        wv[:, :C, :] = -2.0 * y[:, :, sl]
        wv[:, 4, :] = 1.0
        in_maps.append({
            "gk": gk, "mk": mk, "yp": yp_full, "ys": ys_full,
            "wv": wv.astype(bf),
            "wc": np.ascontiguousarray(wcf[:, sl]).astype(bf),
            "sc": sc,
        })
    return in_maps


def kernel(y_hat_softmax, kernel, kernel_xy_mask, kernel_h, kernel_w):
    global _BUILT, LAST_RESULT
    from concourse.bass_utils import run_bass_kernel_spmd

    if _BUILT is None:
        _BUILT = _build()
    nc = _BUILT

    in_maps = _prep_inputs(y_hat_softmax, kernel, kernel_xy_mask)
    res = run_bass_kernel_spmd(nc, in_maps, list(range(NCORES)))
    LAST_RESULT = res
    total = np.float32(0.0)
    for i in range(NCORES):
        total += np.float32(res.results[i]["partial"][0, 0])
    return np.float32(total)


# revision 15
# speedup vs baseline: 1.5026x; 1.5026x over previous
"""AttnGatedCRFV2 loss on 8 Trainium2 NeuronCores.

Math (identical to the reference, no [B,HW,HW] intermediates):
    m    = (1 - eye(HW)) * kernel_xy_mask          (diag folded on host)
    G_b  = kernel_b * m                            (folded on host, fp8 e4m3)
    loss = (1/(HW*B)) * [ sum_b <G_b, 1 - 2*Y_b Y_b^T> + <m, sum_b Y_b Y_b^T> ]
         = (1/(HW*B)) * [ sum_b sum_j ( R1_b[4,j] + sum_c -2*y_b[c,j]*R1_b[c,j] )
                          + sum_j sum_ch wc[ch,j]*R3[ch,j] ]
    where R1_b = [Y_b | 1]^T G_b  (rows c<4 and the ones row 4),
          R3   = Ystack^T m        (Ystack packs all 32 (b,c) channels).

Sharding: j-columns split across 8 cores (288 cols each). Each core reads
G[:, :, j0:j0+288] (5.31 MB in fp8) once; host sums 8 scalar partials.

Device pipeline per core (all heavy traffic fp8):
  - G arrives host-packed as [B, 128, 9, 2, 288]: DoubleRow pair-group s
    holds i-rows 256s+128t+p.  3 chunked DMAs per batch over 3 queues.
  - PE: 9 fp8 DoubleRow matmuls per batch (K=256 each) accumulate
    R1_b = Yext_b^T G_b into a [8, 288] PSUM bank; the resident mask tile
    feeds an identical 9-matmul chain for R3 ([32, 288]).
  - DVE: one fused affine_mul_reduce per bank: accum[p] = sum_j R1[p,j]*W[p,j]
    straight from PSUM (no ACT copy), landing in per-batch columns of a
    small SBUF accumulator.
  - tail: row-reduce the accumulator, two tiny f32 matmuls against a
    SCALE-filled vector produce the [1,1] partial, DMA out.

Host prep is layout/dtype only: fold mask & diag into G, cast fp8/bf16,
pack the DoubleRow layout, fold -2*y into stage-2 weights (as the previous
fp32 baseline already did).  fp8 e4m3 quantization of G/Y changes the loss
by ~2e-6 relative (tolerance 2e-2): errors average out over 42M terms.
"""
import numpy as np
import ml_dtypes
from contextlib import ExitStack

B, C, H, W = 8, 4, 48, 48
HW = H * W                    # 2304
NCORES = 8
JW = HW // NCORES             # 288 j-columns per core
NS = 9                        # DoubleRow pair groups (2304 = 9*2*128)
SCALE = 1.0 / float(HW * B)

_BUILT = None
LAST_RESULT = None


def _build(loop_n=None, gbufs=8, nchunks=1, mode="full", nqueues=1, pack=None, unroll=1):
    from concourse import bacc, tile, mybir

    if pack is None:
        pack = PACK
    f32, bf16 = mybir.dt.float32, mybir.dt.bfloat16
    f8 = mybir.dt.float8e4
    DR = mybir.MatmulPerfMode.DoubleRow

    nc = bacc.Bacc("TRN2", target_bir_lowering=False, debug=False,
                   num_devices=NCORES)

    BP = B // pack
    if mode == "dmaflat":
        g_ap = nc.dram_tensor("gk", [BP, 128, pack * NS * 2 * JW], f8,
                              kind="ExternalInput").ap()
    else:
        g_ap = nc.dram_tensor("gk", [BP, 128, pack, NS, 2, JW], f8,
                              kind="ExternalInput").ap()
    m_ap = nc.dram_tensor("mk", [128, NS, 2, JW], f8, kind="ExternalInput").ap()
    yp_ap = nc.dram_tensor("yp", [B, 128, NS, 2, 16], f8, kind="ExternalInput").ap()
    ys_ap = nc.dram_tensor("ys", [128, NS, 2, 32], f8, kind="ExternalInput").ap()
    wv_ap = nc.dram_tensor("wv", [B, 16, JW], bf16, kind="ExternalInput").ap()
    wc_ap = nc.dram_tensor("wc", [32, JW], bf16, kind="ExternalInput").ap()
    sc_ap = nc.dram_tensor("sc", [32, 1], f32, kind="ExternalInput").ap()
    out_ap = nc.dram_tensor("partial", [1, 1], f32, kind="ExternalOutput").ap()

    qs = [nc.gpsimd, nc.sync, nc.scalar][:nqueues]

    with tile.TileContext(nc) as tc, ExitStack() as ctx:
        consts = ctx.enter_context(tc.tile_pool(name="consts", bufs=1))
        gp = ctx.enter_context(tc.tile_pool(name="gp", bufs=gbufs))
        scr = ctx.enter_context(tc.tile_pool(name="scr", bufs=2))
        smp = ctx.enter_context(tc.tile_pool(name="smp", bufs=2))
        psb = ctx.enter_context(tc.tile_pool(name="psb", bufs=4, space="PSUM"))
        psc = ctx.enter_context(tc.tile_pool(name="psc", bufs=2, space="PSUM"))
        psf = ctx.enter_context(tc.tile_pool(name="psf", bufs=2, space="PSUM"))

        # ---- resident constants ----
        mt = consts.tile([128, NS, 2, JW], f8, tag="mt")
        nc.sync.dma_start(mt[:], m_ap[:])
        yp = {}
        for b in range(B):
            t = consts.tile([128, NS, 2, 16], f8, tag=f"yp{b}")
            nc.sync.dma_start(t[:], yp_ap[b])
            yp[b] = t
        ys = consts.tile([128, NS, 2, 32], f8, tag="ys")
        nc.sync.dma_start(ys[:], ys_ap[:])
        wv = {}
        for b in range(B):
            t = consts.tile([16, JW], bf16, tag=f"wv{b}")
            nc.sync.dma_start(t[:], wv_ap[b])
            wv[b] = t
        wct = consts.tile([32, JW], bf16, tag="wct")
        nc.sync.dma_start(wct[:], wc_ap[:])
        sct = consts.tile([32, 1], f32, tag="sct")
        nc.sync.dma_start(sct[:], sc_ap[:])

        def body():
            # G tile DMAs first so all queues start streaming immediately
            gt = {}
            if mode == "dmaflat":
                for bp in range(B // pack):
                    t = gp.tile([128, pack * NS * 2 * JW], f8, tag="g",
                                name=f"g{bp}")
                    qs[bp % len(qs)].dma_start(t[:], g_ap[bp])
                sc_out = smp.tile([1, 1], f32, tag="sc_out", name="sc_out")
                nc.scalar.copy(sc_out[:], sct[0:1, :])
                nc.sync.dma_start(out_ap[:, :], sc_out[:])
                return
            for bp in range(B // pack):
                if mode == "compute":
                    for i in range(pack):
                        gt[bp * pack + i] = (mt, None)
                    continue
                t = gp.tile([128, pack, NS, 2, JW], f8, tag="g", name=f"g{bp}")
                for ch in range(nchunks):
                    i0 = ch * (pack // nchunks)
                    i1 = i0 + pack // nchunks
                    qs[(bp * nchunks + ch) % len(qs)].dma_start(
                        t[:, i0:i1, :, :, :], g_ap[bp, :, i0:i1, :, :, :])
                for i in range(pack):
                    gt[bp * pack + i] = (t, i)

            if mode == "dma":
                sc_out = smp.tile([1, 1], f32, tag="sc_out", name="sc_out")
                nc.scalar.copy(sc_out[:], sct[0:1, :])
                nc.sync.dma_start(out_ap[:, :], sc_out[:])
                return

            # C chain on the resident mask: R3 = Ystack^T m
            pc = psc.tile([32, JW], f32, tag="pc", name="pc")
            for s in range(NS):
                nc.tensor.matmul(out=pc[:, :], lhsT=ys[:, s, :, :],
                                 rhs=mt[:, s, :, :], perf_mode=DR,
                                 start=(s == 0), stop=(s == NS - 1))
            outc = scr.tile([32, JW], bf16, tag="outc", name="outc")
            accC = smp.tile([32, 1], f32, tag="accC", name="accC")
            nc.vector.affine_mul_reduce(out=outc[:], accum_out=accC[:],
                                        in0=pc[:, :], in1=wct[:], scale=1.0,
                                        bias=0.0)

            # main: R1_b = Yext_b^T G_b, then fused product+reduce per batch
            accA = smp.tile([16, B], f32, tag="accA", name="accA")
            for b in range(B):
                pb = psb.tile([16, JW], f32, tag="pb", name=f"pb{b}")
                tb_, ib_ = gt[b]
                for s in range(NS):
                    rhs = (tb_[:, s, :, :] if ib_ is None
                           else tb_[:, ib_, s, :, :])
                    nc.tensor.matmul(out=pb[:, :], lhsT=yp[b][:, s, :, :],
                                     rhs=rhs, perf_mode=DR,
                                     start=(s == 0), stop=(s == NS - 1))
                outb = scr.tile([16, JW], bf16, tag="outb", name=f"outb{b}")
                nc.vector.affine_mul_reduce(out=outb[:], accum_out=accA[:, b : b + 1],
                                            in0=pb[:, :], in1=wv[b][:], scale=1.0,
                                            bias=0.0)

            # tail: reduce accA cols, contract both accs against SCALE vector
            accAr = smp.tile([16, 1], f32, tag="accAr", name="accAr")
            nc.vector.tensor_reduce(out=accAr[:], in_=accA[:],
                                    axis=mybir.AxisListType.X,
                                    op=mybir.AluOpType.add)
            fin = psf.tile([1, 1], f32, tag="fin", name="fin")
            nc.tensor.matmul(out=fin[:], lhsT=accAr[:], rhs=sct[0:16, :],
                             start=True, stop=False)
            nc.tensor.matmul(out=fin[:], lhsT=accC[:], rhs=sct[:, :],
                             start=False, stop=True)
            sc_out = smp.tile([1, 1], f32, tag="sc_out", name="sc_out")
            nc.scalar.copy(sc_out[:], fin[:])
            nc.sync.dma_start(out_ap[:, :], sc_out[:])

        if loop_n is None:
            for _ in range(unroll):
                body()
        else:
            with tc.For_i(0, loop_n, 1):
                for _ in range(unroll):
                    body()

    nc.compile()
    return nc


PACK = 4


def _prep_inputs(y_hat_softmax, kern, mask, pack=None):
    if pack is None:
        pack = PACK
    e4 = ml_dtypes.float8_e4m3
    bf = ml_dtypes.bfloat16
    y = np.ascontiguousarray(np.asarray(y_hat_softmax, np.float32)).reshape(B, C, HW)
    kern = np.asarray(kern, np.float32)
    m = np.asarray(mask, np.float32) * (1.0 - np.eye(HW, dtype=np.float32))

    G8 = (kern * m[None]).astype(e4)                 # [B, HW, HW]
    m8 = m.astype(e4)                                # exact 0/1 in fp8
    yk = y.transpose(0, 2, 1)                        # [B, HW, C]
    yx = np.zeros((B, HW, 16), np.float32)
    yx[:, :, :C] = yk
    yx[:, :, 4] = 1.0
    yp_full = np.ascontiguousarray(
        yx.reshape(B, NS, 2, 128, 16).transpose(0, 3, 1, 2, 4)).astype(e4)
    yst = np.ascontiguousarray(yk.transpose(1, 0, 2)).reshape(HW, 32)
    ys_full = np.ascontiguousarray(
        yst.reshape(NS, 2, 128, 32).transpose(2, 0, 1, 3)).astype(e4)
    wcf = y.reshape(32, HW)
    sc = np.full((32, 1), SCALE, np.float32)

    in_maps = []
    for cid in range(NCORES):
        j0 = JW * cid
        sl = slice(j0, j0 + JW)
        gk = np.ascontiguousarray(
            G8[:, :, sl].reshape(B, NS, 2, 128, JW).transpose(0, 3, 1, 2, 4))
        gk = np.ascontiguousarray(
            gk.reshape(B // pack, pack, 128, NS, 2, JW).transpose(0, 2, 1, 3, 4, 5))
        mk = np.ascontiguousarray(
            m8[:, sl].reshape(NS, 2, 128, JW).transpose(2, 0, 1, 3))
        wv = np.zeros((B, 16, JW), np.float32)
        wv[:, :C, :] = -2.0 * y[:, :, sl]
        wv[:, 4, :] = 1.0
        in_maps.append({
            "gk": gk, "mk": mk, "yp": yp_full, "ys": ys_full,
            "wv": wv.astype(bf),
            "wc": np.ascontiguousarray(wcf[:, sl]).astype(bf),
            "sc": sc,
        })
    return in_maps


def kernel(y_hat_softmax, kernel, kernel_xy_mask, kernel_h, kernel_w):
    global _BUILT, LAST_RESULT
    from concourse.bass_utils import run_bass_kernel_spmd

    if _BUILT is None:
        _BUILT = _build(pack=PACK)
    nc = _BUILT

    in_maps = _prep_inputs(y_hat_softmax, kernel, kernel_xy_mask, pack=PACK)
    res = run_bass_kernel_spmd(nc, in_maps, list(range(NCORES)))
    LAST_RESULT = res
    total = np.float32(0.0)
    for i in range(NCORES):
        total += np.float32(res.results[i]["partial"][0, 0])
    return np.float32(total)


# revision 17
# speedup vs baseline: 1.5267x; 1.0161x over previous
"""AttnGatedCRFV2 loss on 8 Trainium2 NeuronCores.

Math (identical to the reference, no [B,HW,HW] intermediates):
    m    = (1 - eye(HW)) * kernel_xy_mask          (diag folded on host)
    G_b  = kernel_b * m                            (folded on host, fp8 e4m3)
    loss = (1/(HW*B)) * [ sum_b <G_b, 1 - 2*Y_b Y_b^T> + <m, sum_b Y_b Y_b^T> ]
         = (1/(HW*B)) * [ sum_b sum_j ( R1_b[4,j] + sum_c -2*y_b[c,j]*R1_b[c,j] )
                          + sum_j sum_ch wc[ch,j]*R3[ch,j] ]
    where R1_b = [Y_b | 1]^T G_b  (rows c<4 and the ones row 4),
          R3   = Ystack^T m        (Ystack packs all 32 (b,c) channels).

Sharding: j-columns split across 8 cores (288 cols each). Each core reads
G[:, :, j0:j0+288] (5.31 MB in fp8) once; host sums 8 scalar partials.

Device pipeline per core (all heavy traffic fp8):
  - G arrives host-packed as [B//4, 128, 4, 9, 2, 288] quads: DoubleRow
    pair-group s holds i-rows 256s+128t+p; 4 batches share one tile so
    DMA lines are 20.7 KB/partition.  Two DMAs per quad (2 sub-batches
    each) on the gpsimd queue; measured single-queue DMA is ~340 GB/s and
    multi-queue or finer chunks are slower.  Constants load on the sync
    queue so the G stream starts at t=0.
  - PE: 9 fp8 DoubleRow matmuls per batch (K=256 each, the weight pair
    stride must be 16-aligned -> 16-col stationary) accumulate
    R1_b = Yext_b^T G_b into a [16, 288] PSUM bank; the resident mask
    tile feeds an identical 9-matmul chain for R3 ([32, 288], warms the
    PE while the first quad streams).
  - DVE: one fused affine_mul_reduce per bank: accum[p] = sum_j R1[p,j]*W[p,j]
    straight from PSUM (no ACT copy), landing in per-batch columns of a
    small SBUF accumulator.
  - tail: row-reduce the accumulator, two tiny f32 matmuls against a
    SCALE-filled vector produce the [1,1] partial, DMA out.

Measured (loop-in-NEFF differential, unroll=8, axon/PJRT): 16.4-16.6 us
per body vs a 15.8-16.0 us DMA-only floor (~340 GB/s/core); the fp32
baseline was 53.1 us.  Engine budget: DMA 15.8, PE ~11, DVE ~5.

Host prep is layout/dtype only: fold mask & diag into G, cast fp8/bf16,
pack the DoubleRow layout, fold -2*y into stage-2 weights (as the previous
fp32 baseline already did).  fp8 e4m3 quantization of G/Y changes the loss
by ~2e-6 relative (tolerance 2e-2): errors average out over 42M terms.
"""
import numpy as np
import ml_dtypes
from contextlib import ExitStack

B, C, H, W = 8, 4, 48, 48
HW = H * W                    # 2304
NCORES = 8
JW = HW // NCORES             # 288 j-columns per core
NS = 9                        # DoubleRow pair groups (2304 = 9*2*128)
SCALE = 1.0 / float(HW * B)

_BUILT = None
LAST_RESULT = None


def _build(loop_n=None, gbufs=8, nchunks=2, mode="full", nqueues=1, pack=None, unroll=1):
    from concourse import bacc, tile, mybir

    if pack is None:
        pack = PACK
    f32, bf16 = mybir.dt.float32, mybir.dt.bfloat16
    f8 = mybir.dt.float8e4
    DR = mybir.MatmulPerfMode.DoubleRow

    nc = bacc.Bacc("TRN2", target_bir_lowering=False, debug=False,
                   num_devices=NCORES)

    BP = B // pack
    if mode == "dmaf32":
        g_ap = nc.dram_tensor("gk", [BP, 128, pack * NS * JW // 2], f32,
                              kind="ExternalInput").ap()
    elif mode == "dmaflat":
        g_ap = nc.dram_tensor("gk", [BP, 128, pack * NS * 2 * JW], f8,
                              kind="ExternalInput").ap()
    else:
        g_ap = nc.dram_tensor("gk", [BP, 128, pack, NS, 2, JW], f8,
                              kind="ExternalInput").ap()
    m_ap = nc.dram_tensor("mk", [128, NS, 2, JW], f8, kind="ExternalInput").ap()
    yp_ap = nc.dram_tensor("yp", [B, 128, NS, 2, 16], f8, kind="ExternalInput").ap()
    ys_ap = nc.dram_tensor("ys", [128, NS, 2, 32], f8, kind="ExternalInput").ap()
    wv_ap = nc.dram_tensor("wv", [B, 16, JW], bf16, kind="ExternalInput").ap()
    wc_ap = nc.dram_tensor("wc", [32, JW], bf16, kind="ExternalInput").ap()
    sc_ap = nc.dram_tensor("sc", [32, 1], f32, kind="ExternalInput").ap()
    out_ap = nc.dram_tensor("partial", [1, 1], f32, kind="ExternalOutput").ap()

    qs = [nc.gpsimd, nc.sync, nc.scalar][:nqueues]

    with tile.TileContext(nc) as tc, ExitStack() as ctx:
        consts = ctx.enter_context(tc.tile_pool(name="consts", bufs=1))
        gp = ctx.enter_context(tc.tile_pool(name="gp", bufs=gbufs))
        scr = ctx.enter_context(tc.tile_pool(name="scr", bufs=2))
        smp = ctx.enter_context(tc.tile_pool(name="smp", bufs=2))
        psb = ctx.enter_context(tc.tile_pool(name="psb", bufs=4, space="PSUM"))
        psc = ctx.enter_context(tc.tile_pool(name="psc", bufs=2, space="PSUM"))
        psf = ctx.enter_context(tc.tile_pool(name="psf", bufs=2, space="PSUM"))

        # ---- resident constants ----
        mt = consts.tile([128, NS, 2, JW], f8, tag="mt")
        nc.sync.dma_start(mt[:], m_ap[:])
        yp = {}
        for b in range(B):
            t = consts.tile([128, NS, 2, 16], f8, tag=f"yp{b}")
            nc.sync.dma_start(t[:], yp_ap[b])
            yp[b] = t
        ys = consts.tile([128, NS, 2, 32], f8, tag="ys")
        nc.sync.dma_start(ys[:], ys_ap[:])
        wv = {}
        for b in range(B):
            t = consts.tile([16, JW], bf16, tag=f"wv{b}")
            nc.sync.dma_start(t[:], wv_ap[b])
            wv[b] = t
        wct = consts.tile([32, JW], bf16, tag="wct")
        nc.sync.dma_start(wct[:], wc_ap[:])
        sct = consts.tile([32, 1], f32, tag="sct")
        nc.sync.dma_start(sct[:], sc_ap[:])

        def body():
            # G tile DMAs first so all queues start streaming immediately
            gt = {}
            if mode in ("dmaflat", "dmaf32"):
                for bp in range(B // pack):
                    if mode == "dmaf32":
                        t = gp.tile([128, pack * NS * JW // 2], f32, tag="g",
                                    name=f"g{bp}")
                    else:
                        t = gp.tile([128, pack * NS * 2 * JW], f8, tag="g",
                                    name=f"g{bp}")
                    qs[bp % len(qs)].dma_start(t[:], g_ap[bp])
                sc_out = smp.tile([1, 1], f32, tag="sc_out", name="sc_out")
                nc.scalar.copy(sc_out[:], sct[0:1, :])
                nc.sync.dma_start(out_ap[:, :], sc_out[:])
                return
            for bp in range(B // pack):
                if mode == "compute":
                    for i in range(pack):
                        gt[bp * pack + i] = (mt, None)
                    continue
                t = gp.tile([128, pack, NS, 2, JW], f8, tag="g", name=f"g{bp}")
                for ch in range(nchunks):
                    i0 = ch * (pack // nchunks)
                    i1 = i0 + pack // nchunks
                    qs[(bp * nchunks + ch) % len(qs)].dma_start(
                        t[:, i0:i1, :, :, :], g_ap[bp, :, i0:i1, :, :, :])
                for i in range(pack):
                    gt[bp * pack + i] = (t, i)

            if mode == "dma":
                sc_out = smp.tile([1, 1], f32, tag="sc_out", name="sc_out")
                nc.scalar.copy(sc_out[:], sct[0:1, :])
                nc.sync.dma_start(out_ap[:, :], sc_out[:])
                return

            # C chain on the resident mask: R3 = Ystack^T m
            pc = psc.tile([32, JW], f32, tag="pc", name="pc")
            for s in range(NS):
                nc.tensor.matmul(out=pc[:, :], lhsT=ys[:, s, :, :],
                                 rhs=mt[:, s, :, :], perf_mode=DR,
                                 start=(s == 0), stop=(s == NS - 1))
            outc = scr.tile([32, JW], bf16, tag="outc", name="outc")
            accC = smp.tile([32, 1], f32, tag="accC", name="accC")
            nc.vector.affine_mul_reduce(out=outc[:], accum_out=accC[:],
                                        in0=pc[:, :], in1=wct[:], scale=1.0,
                                        bias=0.0)

            # main: R1_b = Yext_b^T G_b, then fused product+reduce per batch
            accA = smp.tile([16, B], f32, tag="accA", name="accA")
            for b in range(B):
                pb = psb.tile([16, JW], f32, tag="pb", name=f"pb{b}")
                tb_, ib_ = gt[b]
                for s in range(NS):
                    rhs = (tb_[:, s, :, :] if ib_ is None
                           else tb_[:, ib_, s, :, :])
                    nc.tensor.matmul(out=pb[:, :], lhsT=yp[b][:, s, :, :],
                                     rhs=rhs, perf_mode=DR,
                                     start=(s == 0), stop=(s == NS - 1))
                outb = scr.tile([16, JW], bf16, tag="outb", name=f"outb{b}")
                nc.vector.affine_mul_reduce(out=outb[:], accum_out=accA[:, b : b + 1],
                                            in0=pb[:, :], in1=wv[b][:], scale=1.0,
                                            bias=0.0)

            # tail: reduce accA cols, contract both accs against SCALE vector
            accAr = smp.tile([16, 1], f32, tag="accAr", name="accAr")
            nc.vector.tensor_reduce(out=accAr[:], in_=accA[:],
                                    axis=mybir.AxisListType.X,
                                    op=mybir.AluOpType.add)
            fin = psf.tile([1, 1], f32, tag="fin", name="fin")
            nc.tensor.matmul(out=fin[:], lhsT=accAr[:], rhs=sct[0:16, :],
                             start=True, stop=False)
            nc.tensor.matmul(out=fin[:], lhsT=accC[:], rhs=sct[:, :],
                             start=False, stop=True)
            sc_out = smp.tile([1, 1], f32, tag="sc_out", name="sc_out")
            nc.scalar.copy(sc_out[:], fin[:])
            nc.sync.dma_start(out_ap[:, :], sc_out[:])

        if loop_n is None:
            for _ in range(unroll):
                body()
        else:
            with tc.For_i(0, loop_n, 1):
                for _ in range(unroll):
                    body()

    nc.compile()
    return nc


PACK = 4


def _prep_inputs(y_hat_softmax, kern, mask, pack=None):
    if pack is None:
        pack = PACK
    e4 = ml_dtypes.float8_e4m3
    bf = ml_dtypes.bfloat16
    y = np.ascontiguousarray(np.asarray(y_hat_softmax, np.float32)).reshape(B, C, HW)
    kern = np.asarray(kern, np.float32)
    m = np.asarray(mask, np.float32) * (1.0 - np.eye(HW, dtype=np.float32))

    G8 = (kern * m[None]).astype(e4)                 # [B, HW, HW]
    m8 = m.astype(e4)                                # exact 0/1 in fp8
    yk = y.transpose(0, 2, 1)                        # [B, HW, C]
    yx = np.zeros((B, HW, 16), np.float32)
    yx[:, :, :C] = yk
    yx[:, :, 4] = 1.0
    yp_full = np.ascontiguousarray(
        yx.reshape(B, NS, 2, 128, 16).transpose(0, 3, 1, 2, 4)).astype(e4)
    yst = np.ascontiguousarray(yk.transpose(1, 0, 2)).reshape(HW, 32)
    ys_full = np.ascontiguousarray(
        yst.reshape(NS, 2, 128, 32).transpose(2, 0, 1, 3)).astype(e4)
    wcf = y.reshape(32, HW)
    sc = np.full((32, 1), SCALE, np.float32)

    in_maps = []
    for cid in range(NCORES):
        j0 = JW * cid
        sl = slice(j0, j0 + JW)
        gk = np.ascontiguousarray(
            G8[:, :, sl].reshape(B, NS, 2, 128, JW).transpose(0, 3, 1, 2, 4))
        gk = np.ascontiguousarray(
            gk.reshape(B // pack, pack, 128, NS, 2, JW).transpose(0, 2, 1, 3, 4, 5))
        mk = np.ascontiguousarray(
            m8[:, sl].reshape(NS, 2, 128, JW).transpose(2, 0, 1, 3))
        wv = np.zeros((B, 16, JW), np.float32)
        wv[:, :C, :] = -2.0 * y[:, :, sl]
        wv[:, 4, :] = 1.0
        in_maps.append({
            "gk": gk, "mk": mk, "yp": yp_full, "ys": ys_full,
            "wv": wv.astype(bf),
            "wc": np.ascontiguousarray(wcf[:, sl]).astype(bf),
            "sc": sc,
        })
    return in_maps


def kernel(y_hat_softmax, kernel, kernel_xy_mask, kernel_h, kernel_w):
    global _BUILT, LAST_RESULT
    from concourse.bass_utils import run_bass_kernel_spmd

    if _BUILT is None:
        _BUILT = _build(pack=PACK)
    nc = _BUILT

    in_maps = _prep_inputs(y_hat_softmax, kernel, kernel_xy_mask, pack=PACK)
    res = run_bass_kernel_spmd(nc, in_maps, list(range(NCORES)))
    LAST_RESULT = res
    total = np.float32(0.0)
    for i in range(NCORES):
        total += np.float32(res.results[i]["partial"][0, 0])
    return np.float32(total)
